# revision 27
# baseline (speedup 1.0000x reference)
"""DiffMoE MLP (8 experts, capacity 1.0) — expert-parallel across 8 TRN2 NeuronCores.

Contract: kernel(**full_inputs) -> full output (4, 2048, 1024) f32.

Strategy (expert-parallel, sharding_hint):
  host   : gating scores + per-expert top-k (bit-identical jnp ops to the
           reference), token gather + fp32 LayerNorm, weight re-layout and
           fp8 hi/lo decomposition, final topk-weight scale + scatter-add.
  device : core e owns expert e. Every GEMM runs as fp8e4 DoubleRow
           (0.5 cyc/row, 256-deep contraction); precision is recovered with
           a 3-term hi/lo product expansion and a linear-path split:

             A@B ~ Ah@Bh + Ah@Bl + Al@Bh    (lo*lo term provably negligible)
             gelu(h) = alpha*h + beta + g(h)
             o = W2 g(h) + alpha*(W2 W1) y + c

           - fc1 (h = W1 y): hi/lo expansion, 12 DR matmuls per f-block
             (~13-bit effective precision at 0.75x the fp16 cost).
           - nonlinear residue g is small and zero-mean: single fp8 pass.
           - linear path M = alpha*W2@W1 (1/4 of fc2's FLOPs): hi/lo
             expansion.
           All operand carriers are pre-scaled by powers of 2 so every
           product lands in the same x256 PSUM domain — one accumulation
           group per tile, epilogue scale 1/256.

           PE work/core: fc1 196608 + fc2 (M 49152 + g 65536) = 311296 cyc
           vs 524288 all-bf16.
"""

import sys

for _p in ("/opt/trn_rl_repo", "/root/.axon_site/_ro/trn_rl_repo"):
    if _p not in sys.path:
        sys.path.append(_p)

import numpy as np
import ml_dtypes

import concourse.bass as bass
import concourse.bacc as bacc
import concourse.tile as tile
from concourse import mybir
from concourse.bass_utils import run_bass_kernel_spmd

E4M3 = ml_dtypes.float8_e4m3

D = 1024          # embed dim
F = 4096          # hidden dim
N_EXP = 8         # experts == cores
BS = 8192         # tokens
K_TOK = 1024      # tokens kept per expert
LN_EPS = 1e-5

P = 128
KD = D // P       # 8   d-chunks
KD2 = KD // 2     # 4   paired d-chunks (DoubleRow)
KF = F // P       # 32  f-chunks
KF2 = KF // 2     # 16  paired f-chunks (DoubleRow)
TH = 512          # moving free dim per matmul (one PSUM bank)
NT = K_TOK // TH  # 2   token halves

ALPHA = 0.5002    # lsq fit of gelu ~ alpha*h + beta over h ~ N(0,1)
BETA = 0.2819
SDOM = 256.0      # shared PSUM domain: every fp8 product carries x256

_NC_CACHE = {}


def _build_nc(debug=False, reps=1, warmup=0):
    nc = bacc.Bacc("TRN2", target_bir_lowering=False, debug=debug)
    f32 = mybir.dt.float32
    f8 = mybir.dt.float8e4

    y8 = nc.dram_tensor("y8", [2, KD2, P, 2 * K_TOK], f8, kind="ExternalInput")
    w1q = nc.dram_tensor("w1q", [KF, P, 2 * KD2 * 2 * P], f8, kind="ExternalInput")
    w2q = nc.dram_tensor("w2q", [KF2, P, 2 * KD * P], f8, kind="ExternalInput")
    msq = nc.dram_tensor("msq", [2, KD2, P, KD * 2 * P], f8, kind="ExternalInput")
    b1r = nc.dram_tensor("b1r", [P, KF], f32, kind="ExternalInput")
    c2r = nc.dram_tensor("c2r", [P, KF], f32, kind="ExternalInput")
    cr = nc.dram_tensor("cr", [P, KD], f32, kind="ExternalInput")
    ot = nc.dram_tensor("ot", [D, K_TOK], f32, kind="ExternalOutput")

    DR = mybir.MatmulPerfMode.DoubleRow

    with tile.TileContext(nc) as tc:
        with (
            tc.tile_pool(name="singles", bufs=1) as singles,
            tc.tile_pool(name="big", bufs=1) as big,
            tc.tile_pool(name="w1p", bufs=12) as w1p,
            tc.tile_pool(name="t1p", bufs=6) as t1p,
            tc.tile_pool(name="t2p", bufs=6) as t2p,
            tc.tile_pool(name="outp", bufs=6) as outp,
            tc.tile_pool(name="psum", bufs=8, space="PSUM") as psum,
        ):
          for _rep in range(reps):
            # ---- PE pstate warmup: dependency-free dummy matmuls occupy
            # the PE during the DMA prologue so the 3us ramp-to-full-clock
            # completes before the first real matmul ----
            if _rep == 0 and warmup:
                dum_w = singles.tile([P, 2, P], f8, name="dumw")
                nc.vector.memset(dum_w, 0)
                dum_y = singles.tile([P, 2, TH], f8, name="dumy")
                nc.vector.memset(dum_y, 0)
                dps = psum.tile([P, TH], mybir.dt.float32, tag="ps", name="dps")
                for _i in range(warmup):
                    nc.tensor.matmul(
                        dps[:, 0:384], dum_w, dum_y[:, :, 0:384],
                        start=True, stop=True, perf_mode=DR,
                    )

            # ---- prologue: first fc1 weight stripe split across both
            # HWDGE queues, hi-tokens right behind; lo-tokens (needed a
            # few blocks later) ride the software-DGE queue ----
            w1_pre = w1p.tile([P, 2, KD2, 2, P], f8, name="w1pre")
            half = 2 * KD2 * 2 * P // 2
            nc.sync.dma_start(out=w1_pre[:, 0, :, :, :], in_=w1q[0, :, 0:half])
            nc.scalar.dma_start(out=w1_pre[:, 1, :, :, :], in_=w1q[0, :, half:])

            y8_sb = big.tile([P, 2, KD2, 2, K_TOK], f8)
            yq = (nc.scalar, nc.sync)
            for k2 in range(KD2):
                yq[k2 % 2].dma_start(
                    out=y8_sb[:, 0, k2, :, :], in_=y8[0, k2])
            for k2 in range(KD2):
                nc.gpsimd.dma_start(
                    out=y8_sb[:, 1, k2, :, :], in_=y8[1, k2])
            # ---- small constants (latency-tolerant, keep off HWDGE) ----
            b1_sb = singles.tile([P, KF], f32)
            nc.gpsimd.dma_start(out=b1_sb, in_=b1r[:])
            c2_sb = singles.tile([P, KF], f32)
            nc.gpsimd.dma_start(out=c2_sb, in_=c2r[:])
            cc_sb = singles.tile([P, KD], f32)
            nc.gpsimd.dma_start(out=cc_sb, in_=cr[:])

            # ---- fc1: 256*h accumulates hh + lh + hl fp8-DR products;
            # epilogue computes the zero-mean gelu residue
            # g = gelu(ps/256 + b1) - alpha*(ps/256) - kappa -> fp8 ----
            g8_sb = big.tile([P, KF, K_TOK], f8)
            w2_sb = big.tile([P, KF2, 2, KD, P], f8)
            ms_sb = big.tile([P, 2, KD2, KD, 2, P], f8)
            for hl in range(2):
                for k2 in range(KD2):
                    nc.gpsimd.dma_start(
                        out=ms_sb[:, hl, k2, :, :, :], in_=msq[hl, k2])
            for m in range(KF):
                if m == 0:
                    w1t = w1_pre
                else:
                    w1t = w1p.tile([P, 2, KD2, 2, P], f8)
                    eng = nc.sync if m % 2 == 0 else nc.scalar
                    eng.dma_start(out=w1t, in_=w1q[m])
                pss = [psum.tile([P, TH], f32, tag="ps",
                                 name=f"ps1_{m}_{t}") for t in range(NT)]
                # (w-part, y-part): hi*hi, lo*hi, then hi*lo last so the
                # late-arriving lo tokens never gate the start
                groups = ((0, 0), (1, 0), (0, 1))
                for gi, (whl, yhl) in enumerate(groups):
                    for k2 in range(KD2):
                        for t in range(NT):
                            # consecutive matmuls share the stationary block
                            nc.tensor.matmul(
                                pss[t], w1t[:, whl, k2, :, :],
                                y8_sb[:, yhl, k2, :, t * TH:(t + 1) * TH],
                                start=(gi == 0 and k2 == 0),
                                stop=(gi == 2 and k2 == KD2 - 1),
                                perf_mode=DR,
                            )
                for t in range(NT):
                    t1 = t1p.tile([P, TH], f32)
                    nc.scalar.activation(
                        t1, pss[t], mybir.ActivationFunctionType.Gelu_apprx_tanh,
                        bias=b1_sb[:, m:m + 1], scale=1.0 / SDOM,
                    )
                    t2 = t2p.tile([P, TH], f32)
                    nc.vector.tensor_scalar(
                        t2, pss[t], -ALPHA / SDOM, c2_sb[:, m:m + 1],
                        mybir.AluOpType.mult, mybir.AluOpType.add,
                    )
                    nc.vector.tensor_tensor(
                        g8_sb[:, m, t * TH:(t + 1) * TH], t1, t2,
                        mybir.AluOpType.add,
                    )
                # stream the resident fc2 fp8 weights during the fc1 loop
                if m % 2 == 0:
                    nc.gpsimd.dma_start(out=w2_sb[:, m // 2], in_=w2q[m // 2])

            # ---- fc2: one PSUM group per (d-block, t): 12 hi/lo linear-path
            # DR products (alpha*W2W1 y) + 16 residue DR products (W2 g);
            # epilogue scales 1/256 and adds the constant fold ----
            for msr in (range(0, 2), range(2, 4), range(4, 6), range(6, 8)):
                ps2 = {(m, t): psum.tile([P, TH], f32, tag="ps",
                                         name=f"ps2_{m}_{t}")
                       for m in msr for t in range(NT)}
                for m in msr:
                    groups = ((0, 0), (1, 0), (0, 1))
                    for gi, (whl, yhl) in enumerate(groups):
                        for k2 in range(KD2):
                            mblk = ms_sb[:, whl, k2, m, :, :]
                            for t in range(NT):
                                nc.tensor.matmul(
                                    ps2[(m, t)], mblk,
                                    y8_sb[:, yhl, k2, :, t * TH:(t + 1) * TH],
                                    start=(gi == 0 and k2 == 0), stop=False,
                                    perf_mode=DR,
                                )
                    for c in range(KF2):
                        w2blk = w2_sb[:, c, :, m, :]
                        for t in range(NT):
                            nc.tensor.matmul(
                                ps2[(m, t)], w2blk,
                                g8_sb[:, 2 * c:2 * c + 2, t * TH:(t + 1) * TH],
                                start=False, stop=(c == KF2 - 1),
                                perf_mode=DR,
                            )
                    for t in range(NT):
                        o_t = outp.tile([P, TH], f32)
                        nc.scalar.activation(
                            o_t, ps2[(m, t)],
                            mybir.ActivationFunctionType.Identity,
                            bias=cc_sb[:, m:m + 1], scale=1.0 / SDOM,
                        )
                        # split the store across both queues to shrink the
                        # exposed tail of the final tile
                        h_ = TH // 2
                        e0 = nc.sync if (m + t) % 2 == 0 else nc.scalar
                        e1 = nc.scalar if (m + t) % 2 == 0 else nc.sync
                        e0.dma_start(
                            out=ot[m * P:(m + 1) * P, t * TH:t * TH + h_],
                            in_=o_t[:, 0:h_],
                        )
                        e1.dma_start(
                            out=ot[m * P:(m + 1) * P, t * TH + h_:(t + 1) * TH],
                            in_=o_t[:, h_:],
                        )

    nc.compile()
    return nc


def get_nc():
    if "nc" not in _NC_CACHE:
        _NC_CACHE["nc"] = _build_nc()
    return _NC_CACHE["nc"]


def _gate_topk(xf32, gate_w):
    """Replicates the reference gating bit-exactly (same jnp ops, same backend)."""
    import jax
    import jax.numpy as jnp

    xf = jnp.asarray(xf32)
    gw = jnp.asarray(np.asarray(gate_w, dtype=np.float32))
    scores = xf @ gw.T
    scores = (jnp.tanh(scores) + 1.0) * 0.5
    vals, idx = jax.lax.top_k(scores.T, K_TOK)   # (n, k)
    return np.asarray(vals), np.asarray(idx)


def _q8(a):
    return a.astype(E4M3)


def _dr_tok(yT):
    """[D, K] value layout -> [KD2, P, 2*K] DoubleRow moving layout
    (d = k2*256 + i*128 + p)."""
    return np.ascontiguousarray(
        yT.reshape(KD2, 2, P, K_TOK).transpose(0, 2, 1, 3)
    ).reshape(KD2, P, 2 * K_TOK)


def _dr_w1(W):
    """[F, D] -> [KF, P, KD2, 2, P] DR stationary layout
    ([m, p, k2, i, f] = W[m*128+f, k2*256+i*128+p])."""
    return np.ascontiguousarray(
        W.reshape(KF, P, KD2, 2, P).transpose(0, 4, 2, 3, 1))


def _dr_m(Mx):
    """[D, D] -> [KD2, P, KD*2*P] DR stationary layout
    ([k2, p, (m, i, dout)] = Mx[m*128+dout, k2*256+i*128+p])."""
    return np.ascontiguousarray(
        Mx.reshape(KD, P, KD2, 2, P).transpose(2, 4, 0, 3, 1)
    ).reshape(KD2, P, KD * 2 * P)


def kernel(x, gate_w, ln_gamma, ln_beta, fc1s, b1s, fc2s, b2s):
    x = np.asarray(x, dtype=np.float32)
    gate_w = np.asarray(gate_w, dtype=np.float32)
    ln_gamma = np.asarray(ln_gamma, dtype=np.float32)
    ln_beta = np.asarray(ln_beta, dtype=np.float32)
    fc1s = np.asarray(fc1s, dtype=np.float32)
    b1s = np.asarray(b1s, dtype=np.float32)
    fc2s = np.asarray(fc2s, dtype=np.float32)
    b2s = np.asarray(b2s, dtype=np.float32)

    og_shape = x.shape
    xf = x.reshape(-1, D)
    vals, idx = _gate_topk(xf, gate_w)

    np_inputs = {"ln_gamma": ln_gamma, "ln_beta": ln_beta,
                 "fc1s": fc1s, "b1s": b1s, "fc2s": fc2s, "b2s": b2s}
    in_maps = build_in_maps(np_inputs, xf, vals, idx)

    nc = get_nc()
    res = run_bass_kernel_spmd(nc, in_maps, core_ids=list(range(N_EXP)))

    out = xf.copy()
    for e in range(N_EXP):
        o_e = np.asarray(res.results[e]["ot"]).T           # (k, d) f32
        out[idx[e]] += o_e * vals[e][:, None]
    return out.reshape(og_shape)


def build_in_maps(np_inputs, xf, vals, idx):
    gam = np_inputs["ln_gamma"]
    bet = np_inputs["ln_beta"]
    maps = []
    for e in range(N_EXP):
        y_e = xf[idx[e]]                                   # (k, d) f32
        mu = y_e.mean(axis=1, keepdims=True)
        var = y_e.var(axis=1, keepdims=True)
        yn = (y_e - mu) / np.sqrt(var + LN_EPS) * gam + bet

        W1 = np_inputs["fc1s"][e]                          # (F, D)
        W2 = np_inputs["fc2s"][e]                          # (D, F)
        b1 = np_inputs["b1s"][e]                           # (F,)
        b2 = np_inputs["b2s"][e]                           # (D,)

        # hi/lo fp8 carriers; every device product lands in the x256 domain
        ynT = np.ascontiguousarray(yn.T)                   # (D, K)
        yh = _q8(4.0 * ynT)
        yl = _q8(4.0 * ynT - yh.astype(np.float32))
        w1h = _q8(64.0 * W1)
        w1l = _q8(64.0 * W1 - w1h.astype(np.float32))
        Mt = ALPHA * (W2 @ W1)                             # (D, D) host fp32
        mh = _q8(64.0 * Mt)
        ml = _q8(64.0 * Mt - mh.astype(np.float32))
        cvec = ALPHA * (W2 @ b1) + BETA * W2.sum(axis=1) + b2

        maps.append({
            "y8": np.stack([_dr_tok(yh), _dr_tok(yl)]),
            "w1q": np.stack([_dr_w1(w1h), _dr_w1(w1l)], axis=2
                            ).reshape(KF, P, 2 * KD2 * 2 * P),
            "w2q": np.ascontiguousarray(
                _q8(SDOM * W2).reshape(KD, P, KF2, 2, P).transpose(2, 4, 3, 0, 1)
            ).reshape(KF2, P, 2 * KD * P),
            "msq": np.stack([_dr_m(mh), _dr_m(ml)]),
            "b1r": np.ascontiguousarray(b1.reshape(KF, P).T),
            "c2r": np.ascontiguousarray(
                (-(ALPHA * b1 + BETA)).reshape(KF, P).T.astype(np.float32)),
            "cr": np.ascontiguousarray(cvec.reshape(KD, P).T.astype(np.float32)),
        })
    return maps


# revision 28
# speedup vs baseline: 1.0332x; 1.0332x over previous
"""DiffMoE MLP (8 experts, capacity 1.0) — expert-parallel across 8 TRN2 NeuronCores.

Contract: kernel(**full_inputs) -> full output (4, 2048, 1024) f32.

Strategy (expert-parallel, sharding_hint):
  host   : gating scores + per-expert top-k (bit-identical jnp ops to the
           reference), token gather + fp32 LayerNorm, weight re-layout and
           fp8 hi/lo decomposition, final topk-weight scale + scatter-add.
  device : core e owns expert e. Every GEMM runs as fp8e4 DoubleRow
           (0.5 cyc/row, 256-deep contraction); precision is recovered with
           a 3-term hi/lo product expansion and a linear-path split:

             A@B ~ Ah@Bh + Ah@Bl + Al@Bh    (lo*lo term provably negligible)
             gelu(h) = alpha*h + beta + g(h)
             o = W2 g(h) + alpha*(W2 W1) y + c

           - fc1 (h = W1 y): hi/lo expansion, 12 DR matmuls per f-block
             (~13-bit effective precision at 0.75x the fp16 cost).
           - nonlinear residue g is small and zero-mean: single fp8 pass.
           - linear path M = alpha*W2@W1 (1/4 of fc2's FLOPs): hi/lo
             expansion.
           All operand carriers are pre-scaled by powers of 2 so every
           product lands in the same x256 PSUM domain — one accumulation
           group per tile, epilogue scale 1/256.

           PE work/core: fc1 196608 + fc2 (M 49152 + g 65536) = 311296 cyc
           vs 524288 all-bf16.
"""

import sys

for _p in ("/opt/trn_rl_repo", "/root/.axon_site/_ro/trn_rl_repo"):
    if _p not in sys.path:
        sys.path.append(_p)

import numpy as np
import ml_dtypes

import concourse.bass as bass
import concourse.bacc as bacc
import concourse.tile as tile
from concourse import mybir
from concourse.bass_utils import run_bass_kernel_spmd

E4M3 = ml_dtypes.float8_e4m3

D = 1024          # embed dim
F = 4096          # hidden dim
N_EXP = 8         # experts == cores
BS = 8192         # tokens
K_TOK = 1024      # tokens kept per expert
LN_EPS = 1e-5

P = 128
KD = D // P       # 8   d-chunks
KD2 = KD // 2     # 4   paired d-chunks (DoubleRow)
KF = F // P       # 32  f-chunks
KF2 = KF // 2     # 16  paired f-chunks (DoubleRow)
TH = 512          # moving free dim per matmul (one PSUM bank)
NT = K_TOK // TH  # 2   token halves

JW1 = 3           # W1-lo correction kept for this many of the 4 d-chunk pairs
ALPHA = 0.5002    # lsq fit of gelu ~ alpha*h + beta over h ~ N(0,1)
BETA = 0.2819
SDOM = 256.0      # shared PSUM domain: every fp8 product carries x256

_NC_CACHE = {}


def _build_nc(debug=False, reps=1, warmup=0):
    nc = bacc.Bacc("TRN2", target_bir_lowering=False, debug=debug)
    f32 = mybir.dt.float32
    f8 = mybir.dt.float8e4

    y8 = nc.dram_tensor("y8", [2, KD2, P, 2 * K_TOK], f8, kind="ExternalInput")
    w1q = nc.dram_tensor("w1q", [KF, P, (KD2 + JW1) * 2 * P], f8, kind="ExternalInput")
    w2q = nc.dram_tensor("w2q", [KF2, P, 2 * KD * P], f8, kind="ExternalInput")
    msq = nc.dram_tensor("msq", [2, KD2, P, KD * 2 * P], f8, kind="ExternalInput")
    b1r = nc.dram_tensor("b1r", [P, KF], f32, kind="ExternalInput")
    c2r = nc.dram_tensor("c2r", [P, KF], f32, kind="ExternalInput")
    cr = nc.dram_tensor("cr", [P, KD], f32, kind="ExternalInput")
    ot = nc.dram_tensor("ot", [D, K_TOK], f32, kind="ExternalOutput")

    DR = mybir.MatmulPerfMode.DoubleRow

    with tile.TileContext(nc) as tc:
        with (
            tc.tile_pool(name="singles", bufs=1) as singles,
            tc.tile_pool(name="big", bufs=1) as big,
            tc.tile_pool(name="w1p", bufs=12) as w1p,
            tc.tile_pool(name="t1p", bufs=6) as t1p,
            tc.tile_pool(name="t2p", bufs=6) as t2p,
            tc.tile_pool(name="outp", bufs=6) as outp,
            tc.tile_pool(name="psum", bufs=8, space="PSUM") as psum,
        ):
          for _rep in range(reps):
            # ---- PE pstate warmup: dependency-free dummy matmuls occupy
            # the PE during the DMA prologue so the 3us ramp-to-full-clock
            # completes before the first real matmul ----
            if _rep == 0 and warmup:
                dum_w = singles.tile([P, 2, P], f8, name="dumw")
                nc.vector.memset(dum_w, 0)
                dum_y = singles.tile([P, 2, TH], f8, name="dumy")
                nc.vector.memset(dum_y, 0)
                dps = psum.tile([P, TH], mybir.dt.float32, tag="ps", name="dps")
                for _i in range(warmup):
                    nc.tensor.matmul(
                        dps[:, 0:384], dum_w, dum_y[:, :, 0:384],
                        start=True, stop=True, perf_mode=DR,
                    )

            # ---- prologue: first fc1 weight stripe split across both
            # HWDGE queues, hi-tokens right behind; lo-tokens (needed a
            # few blocks later) ride the software-DGE queue ----
            w1_pre = w1p.tile([P, KD2 + JW1, 2, P], f8, name="w1pre")
            half = KD2 * 2 * P
            nc.sync.dma_start(out=w1_pre[:, 0:KD2, :, :], in_=w1q[0, :, 0:half])
            nc.scalar.dma_start(out=w1_pre[:, KD2:, :, :], in_=w1q[0, :, half:])

            y8_sb = big.tile([P, 2, KD2, 2, K_TOK], f8)
            yq = (nc.scalar, nc.sync)
            for k2 in range(KD2):
                yq[k2 % 2].dma_start(
                    out=y8_sb[:, 0, k2, :, :], in_=y8[0, k2])
            for k2 in range(KD2):
                nc.gpsimd.dma_start(
                    out=y8_sb[:, 1, k2, :, :], in_=y8[1, k2])
            # ---- small constants (latency-tolerant, keep off HWDGE) ----
            b1_sb = singles.tile([P, KF], f32)
            nc.gpsimd.dma_start(out=b1_sb, in_=b1r[:])
            c2_sb = singles.tile([P, KF], f32)
            nc.gpsimd.dma_start(out=c2_sb, in_=c2r[:])
            cc_sb = singles.tile([P, KD], f32)
            nc.gpsimd.dma_start(out=cc_sb, in_=cr[:])

            # ---- fc1: 256*h accumulates hh + lh + hl fp8-DR products;
            # epilogue computes the zero-mean gelu residue
            # g = gelu(ps/256 + b1) - alpha*(ps/256) - kappa -> fp8 ----
            g8_sb = big.tile([P, KF, K_TOK], f8)
            w2_sb = big.tile([P, KF2, 2, KD, P], f8)
            ms_sb = big.tile([P, 2, KD2, KD, 2, P], f8)
            for hl in range(2):
                for k2 in range(KD2):
                    nc.gpsimd.dma_start(
                        out=ms_sb[:, hl, k2, :, :, :], in_=msq[hl, k2])
            for m in range(KF):
                if m == 0:
                    w1t = w1_pre
                else:
                    w1t = w1p.tile([P, KD2 + JW1, 2, P], f8)
                    eng = nc.sync if m % 2 == 0 else nc.scalar
                    eng.dma_start(out=w1t, in_=w1q[m])
                pss = [psum.tile([P, TH], f32, tag="ps",
                                 name=f"ps1_{m}_{t}") for t in range(NT)]
                # hi*hi (4 chunks), lo*hi (JW1 chunks), then hi*lo last so
                # the late-arriving lo tokens never gate the start
                plan = ([(k2, k2, 0) for k2 in range(KD2)] +
                        [(KD2 + k2, k2, 0) for k2 in range(JW1)] +
                        [(k2, k2, 1) for k2 in range(KD2)])
                for pi, (wc, k2, yhl) in enumerate(plan):
                    for t in range(NT):
                        # consecutive matmuls share the stationary block
                        nc.tensor.matmul(
                            pss[t], w1t[:, wc, :, :],
                            y8_sb[:, yhl, k2, :, t * TH:(t + 1) * TH],
                            start=(pi == 0),
                            stop=(pi == len(plan) - 1),
                            perf_mode=DR,
                        )
                for t in range(NT):
                    t1 = t1p.tile([P, TH], f32)
                    nc.scalar.activation(
                        t1, pss[t], mybir.ActivationFunctionType.Gelu_apprx_tanh,
                        bias=b1_sb[:, m:m + 1], scale=1.0 / SDOM,
                    )
                    t2 = t2p.tile([P, TH], f32)
                    nc.vector.tensor_scalar(
                        t2, pss[t], -ALPHA / SDOM, c2_sb[:, m:m + 1],
                        mybir.AluOpType.mult, mybir.AluOpType.add,
                    )
                    nc.vector.tensor_tensor(
                        g8_sb[:, m, t * TH:(t + 1) * TH], t1, t2,
                        mybir.AluOpType.add,
                    )
                # stream the resident fc2 fp8 weights during the fc1 loop
                if m % 2 == 0:
                    nc.gpsimd.dma_start(out=w2_sb[:, m // 2], in_=w2q[m // 2])

            # ---- fc2: one PSUM group per (d-block, t): 12 hi/lo linear-path
            # DR products (alpha*W2W1 y) + 16 residue DR products (W2 g);
            # epilogue scales 1/256 and adds the constant fold ----
            for msr in (range(0, 2), range(2, 4), range(4, 6), range(6, 8)):
                ps2 = {(m, t): psum.tile([P, TH], f32, tag="ps",
                                         name=f"ps2_{m}_{t}")
                       for m in msr for t in range(NT)}
                for m in msr:
                    groups = ((0, 0), (1, 0), (0, 1))
                    for gi, (whl, yhl) in enumerate(groups):
                        for k2 in range(KD2):
                            mblk = ms_sb[:, whl, k2, m, :, :]
                            for t in range(NT):
                                nc.tensor.matmul(
                                    ps2[(m, t)], mblk,
                                    y8_sb[:, yhl, k2, :, t * TH:(t + 1) * TH],
                                    start=(gi == 0 and k2 == 0), stop=False,
                                    perf_mode=DR,
                                )
                    for c in range(KF2):
                        w2blk = w2_sb[:, c, :, m, :]
                        for t in range(NT):
                            nc.tensor.matmul(
                                ps2[(m, t)], w2blk,
                                g8_sb[:, 2 * c:2 * c + 2, t * TH:(t + 1) * TH],
                                start=False, stop=(c == KF2 - 1),
                                perf_mode=DR,
                            )
                    for t in range(NT):
                        o_t = outp.tile([P, TH], f32)
                        nc.scalar.activation(
                            o_t, ps2[(m, t)],
                            mybir.ActivationFunctionType.Identity,
                            bias=cc_sb[:, m:m + 1], scale=1.0 / SDOM,
                        )
                        # split the store across both queues to shrink the
                        # exposed tail of the final tile
                        h_ = TH // 2
                        e0 = nc.sync if (m + t) % 2 == 0 else nc.scalar
                        e1 = nc.scalar if (m + t) % 2 == 0 else nc.sync
                        e0.dma_start(
                            out=ot[m * P:(m + 1) * P, t * TH:t * TH + h_],
                            in_=o_t[:, 0:h_],
                        )
                        e1.dma_start(
                            out=ot[m * P:(m + 1) * P, t * TH + h_:(t + 1) * TH],
                            in_=o_t[:, h_:],
                        )

    nc.compile()
    return nc


def get_nc():
    if "nc" not in _NC_CACHE:
        _NC_CACHE["nc"] = _build_nc()
    return _NC_CACHE["nc"]


def _gate_topk(xf32, gate_w):
    """Replicates the reference gating bit-exactly (same jnp ops, same backend)."""
    import jax
    import jax.numpy as jnp

    xf = jnp.asarray(xf32)
    gw = jnp.asarray(np.asarray(gate_w, dtype=np.float32))
    scores = xf @ gw.T
    scores = (jnp.tanh(scores) + 1.0) * 0.5
    vals, idx = jax.lax.top_k(scores.T, K_TOK)   # (n, k)
    return np.asarray(vals), np.asarray(idx)


def _q8(a):
    return a.astype(E4M3)


def _dr_tok(yT):
    """[D, K] value layout -> [KD2, P, 2*K] DoubleRow moving layout
    (d = k2*256 + i*128 + p)."""
    return np.ascontiguousarray(
        yT.reshape(KD2, 2, P, K_TOK).transpose(0, 2, 1, 3)
    ).reshape(KD2, P, 2 * K_TOK)


def _dr_w1(W):
    """[F, D] -> [KF, P, KD2, 2, P] DR stationary layout
    ([m, p, k2, i, f] = W[m*128+f, k2*256+i*128+p])."""
    return np.ascontiguousarray(
        W.reshape(KF, P, KD2, 2, P).transpose(0, 4, 2, 3, 1))


def _pack_w1(w1h, w1l):
    """hi chunks (all KD2) then lo chunks (first JW1) -> [KF, P, (KD2+JW1)*2P]."""
    hi = _dr_w1(w1h.astype(np.float32)).astype(E4M3)
    lo = _dr_w1(w1l.astype(np.float32)).astype(E4M3)[:, :, :JW1]
    return np.concatenate([hi, lo], axis=2).reshape(KF, P, (KD2 + JW1) * 2 * P)


def _dr_m(Mx):
    """[D, D] -> [KD2, P, KD*2*P] DR stationary layout
    ([k2, p, (m, i, dout)] = Mx[m*128+dout, k2*256+i*128+p])."""
    return np.ascontiguousarray(
        Mx.reshape(KD, P, KD2, 2, P).transpose(2, 4, 0, 3, 1)
    ).reshape(KD2, P, KD * 2 * P)


def kernel(x, gate_w, ln_gamma, ln_beta, fc1s, b1s, fc2s, b2s):
    x = np.asarray(x, dtype=np.float32)
    gate_w = np.asarray(gate_w, dtype=np.float32)
    ln_gamma = np.asarray(ln_gamma, dtype=np.float32)
    ln_beta = np.asarray(ln_beta, dtype=np.float32)
    fc1s = np.asarray(fc1s, dtype=np.float32)
    b1s = np.asarray(b1s, dtype=np.float32)
    fc2s = np.asarray(fc2s, dtype=np.float32)
    b2s = np.asarray(b2s, dtype=np.float32)

    og_shape = x.shape
    xf = x.reshape(-1, D)
    vals, idx = _gate_topk(xf, gate_w)

    np_inputs = {"ln_gamma": ln_gamma, "ln_beta": ln_beta,
                 "fc1s": fc1s, "b1s": b1s, "fc2s": fc2s, "b2s": b2s}
    in_maps = build_in_maps(np_inputs, xf, vals, idx)

    nc = get_nc()
    res = run_bass_kernel_spmd(nc, in_maps, core_ids=list(range(N_EXP)))

    out = xf.copy()
    for e in range(N_EXP):
        o_e = np.asarray(res.results[e]["ot"]).T           # (k, d) f32
        out[idx[e]] += o_e * vals[e][:, None]
    return out.reshape(og_shape)


def build_in_maps(np_inputs, xf, vals, idx):
    gam = np_inputs["ln_gamma"]
    bet = np_inputs["ln_beta"]
    maps = []
    for e in range(N_EXP):
        y_e = xf[idx[e]]                                   # (k, d) f32
        mu = y_e.mean(axis=1, keepdims=True)
        var = y_e.var(axis=1, keepdims=True)
        yn = (y_e - mu) / np.sqrt(var + LN_EPS) * gam + bet

        W1 = np_inputs["fc1s"][e]                          # (F, D)
        W2 = np_inputs["fc2s"][e]                          # (D, F)
        b1 = np_inputs["b1s"][e]                           # (F,)
        b2 = np_inputs["b2s"][e]                           # (D,)

        # hi/lo fp8 carriers; every device product lands in the x256 domain
        ynT = np.ascontiguousarray(yn.T)                   # (D, K)
        yh = _q8(4.0 * ynT)
        yl = _q8(4.0 * ynT - yh.astype(np.float32))
        w1h = _q8(64.0 * W1)
        w1l = _q8(64.0 * W1 - w1h.astype(np.float32))
        Mt = ALPHA * (W2 @ W1)                             # (D, D) host fp32
        mh = _q8(64.0 * Mt)
        ml = _q8(64.0 * Mt - mh.astype(np.float32))
        cvec = ALPHA * (W2 @ b1) + BETA * W2.sum(axis=1) + b2

        maps.append({
            "y8": np.stack([_dr_tok(yh), _dr_tok(yl)]),
            "w1q": _pack_w1(w1h, w1l),
            "w2q": np.ascontiguousarray(
                _q8(SDOM * W2).reshape(KD, P, KF2, 2, P).transpose(2, 4, 3, 0, 1)
            ).reshape(KF2, P, 2 * KD * P),
            "msq": np.stack([_dr_m(mh), _dr_m(ml)]),
            "b1r": np.ascontiguousarray(b1.reshape(KF, P).T),
            "c2r": np.ascontiguousarray(
                (-(ALPHA * b1 + BETA)).reshape(KF, P).T.astype(np.float32)),
            "cr": np.ascontiguousarray(cvec.reshape(KD, P).T.astype(np.float32)),
        })
    return maps
